# revision 16
# baseline (speedup 1.0000x reference)
"""InterpretableMultiHeadAttention kernel for 8 Trainium2 NeuronCores.

Math (per batch b): q/k = x@Wq/k + b; per-head logits = q_h k_h^T/sqrt(dh);
probs = sparsemax(logits); shared V = head-mean of v (linear -> fold into a
(D, dh) weight); out = concat_h(probs_h @ v_shared) @ Wo + bo;
avg_attention = mean_h probs.

Sharding: core c handles batch b=c//2, head-group g=c%2 (8 of 16 heads).

Wire-format optimization (the metric is dominated by host<->device transfer
over the axon tunnel, ~19 ms/MB each way plus ~10 ms per array):
  * everything on the wire is fp16 (intermediate math stays f32 in PSUM);
  * each core uploads ONE packed blob holding only a disjoint shard of
    x^T / Wq / Wk / Wo / v_shared; full tensors are reassembled on-device
    with two AllGather collectives (pair groups for x, quad groups for the
    weights);
  * the two big outputs are combined by a single pair-ReduceScatter whose
    rank split hands core (b,0) the summed x_out^T[b] and core (b,1) the
    summed avg^T[b], so each core downloads ONE packed blob and the
    head-group partial sums never cross the tunnel;
  * the downloaded payload is row-quantized to int8 on device (scale =
    126/rowmax per row, fp16 row-maxes shipped alongside) -- halves both the
    download and the donated-zero upload for ~0.3% added output error.

Everything on-device is computed transposed (queries on the free axis) so
every matmul consumes natural layouts; the host transposes the two big
outputs at the end.

Sparsemax per query row: top-8 extraction (nc.vector.max reading the logits
PSUM directly, sorted desc), closed-form tau* = max_j (cumsum_j - 1)/j over
the sorted prefix, with the cumsum/argmax math batched across all 8 query
tiles of a head (j-major [128, 64] layout).  Rows whose support size is >= 8
are flagged (z_8 > t_8) and corrected exactly on the host (~1% of rows for
this distribution).  -tau is scattered into row 64 of the per-head augmented
q tile (k tile row 64 holds ones), so a single 65-partition matmul emits
z - tau and the final probs come out of one fused Relu(PSUM) pass.

The runtime charges ~44 us of dispatch overhead per instruction, so the
device program batches scalar work aggressively (~1300 instructions total);
engine time itself is ~0.5 ms (TimelineSim).
"""

import sys

sys.path.insert(0, "/opt/trn_rl_repo")

import jax


def _set_compile_cache(enabled):
    # run_bass_via_pjrt builds a fresh jit(shard_map(...)) per call; the
    # persistent cache lets warm calls load the XLA executable from disk
    # (~150 ms/call here).  Disabled on retry after a tunnel failure so the
    # fallback path recompiles from scratch.
    try:
        jax.config.update("jax_compilation_cache_dir",
                          "/tmp/jax_cc_cache" if enabled else None)
        jax.config.update("jax_persistent_cache_min_compile_time_secs", 0.0)
        jax.config.update("jax_persistent_cache_min_entry_size_bytes", -1)
    except Exception:
        pass


_set_compile_cache(True)

import numpy as np
from contextlib import ExitStack

import concourse.bacc as bacc
import concourse.mybir as mybir
import concourse.tile as tile
from concourse.bass_utils import run_bass_kernel_spmd

F32 = mybir.dt.float32
F16 = mybir.dt.float16
I8 = mybir.dt.int8
AX = mybir.AxisListType
ALU = mybir.AluOpType
ACTF = mybir.ActivationFunctionType

N_CORES = 8
P = 128
B, S, D = 4, 1024, 1024
H = 16                      # total heads
HG = 8                      # heads per core (head-group)
DH = D // H                 # 64
GW = D // 2                 # 512 = per-group projection width
NT = S // P                 # 8 i/j tiles

# input blob row offsets (width 1024, fp16)
RX = 0          # xTh               [512, 1024]
RWQ = 512       # wq   [256,512] -> [128, 1024]
RWK = 640       # wk   [256,512] -> [128, 1024]
RWO = 768       # wo int8 [128,1024] -> [64, 1024] f16-packed
RWOS = 832      # wo row-maxes/126 f16: [1, 1024] (cols 0:128 used)
RWV = 833       # wv   [256, 64] -> [ 16, 1024]
RB = 849        # row: [bq*0.125 | bk]
RBV = 850       # row: [bv_sh pad]
IN_ROWS = 851
WG_ROWS = RB - RWQ          # 337 rows gathered per quad rank

# output blob row offsets (width 1024, int8): rows 0:1024 row-quantized
# payload, 2 rows fp16 row-maxes (bitcast), then tau/flag fp16 (bitcast)
OSCALE = 1024   # rowmax [128, 8] f16 -> [2, 1024] i8
OTAU = 1026     # tau  [128, 64] f16 -> [16, 1024] i8
OFLAG = 1042    # flag [128, 64] f16 -> [16, 1024] i8
OUT_ROWS = 1058
QMAX = 126.0

PAIRS = [[0, 1], [2, 3], [4, 5], [6, 7]]     # {batch} x {head-group g}
QUADS = [[0, 2, 4, 6], [1, 3, 5, 7]]         # same g across batches
_cached = {}


def _build():
    nc = bacc.Bacc("TRN2", target_bir_lowering=False, debug=False,
                   num_devices=N_CORES)

    inb_d = nc.dram_tensor("inb", [IN_ROWS, S], F16, kind="ExternalInput").ap()
    outb_d = nc.dram_tensor("outb", [OUT_ROWS, S], I8, kind="ExternalOutput").ap()

    with tile.TileContext(nc) as tc, ExitStack() as es:
        dram = es.enter_context(tc.tile_pool(name="dram", bufs=1, space="DRAM"))
        sb = es.enter_context(tc.tile_pool(name="persist", bufs=1))
        psZ = es.enter_context(tc.tile_pool(name="psZ", bufs=2, space="PSUM"))
        psB = es.enter_context(tc.tile_pool(name="psB", bufs=2, space="PSUM"))
        psO = es.enter_context(tc.tile_pool(name="psO", bufs=2, space="PSUM"))
        if True:
            # ---- input gather collectives (DRAM bounce buffers) ----
            xg_i = dram.tile([GW, S], F16)
            xg_o = dram.tile([D, S], F16)          # full xT
            wg_i = dram.tile([WG_ROWS, S], F16)
            wg_o = dram.tile([4 * WG_ROWS, S], F16)  # 4 rank blocks of weights

            nc.gpsimd.dma_start(xg_i[:], inb_d[RX:RX + GW, :])
            nc.gpsimd.dma_start(wg_i[:], inb_d[RWQ:RB, :])
            nc.gpsimd.collective_compute(
                "AllGather", ALU.bypass, replica_groups=PAIRS,
                ins=[xg_i.opt()], outs=[xg_o.opt()])
            nc.gpsimd.collective_compute(
                "AllGather", ALU.bypass, replica_groups=QUADS,
                ins=[wg_i.opt()], outs=[wg_o.opt()])

            # ---- constants ----
            ones_r = sb.tile([1, S], F16)
            nc.vector.memset(ones_r[:], 1.0)
            recipJ = sb.tile([P, 64], F32)       # j-major: block j = 1/(j+1)
            for j in range(8):
                nc.vector.memset(recipJ[:, j * 8:(j + 1) * 8], 1.0 / (j + 1))

            # ---- persistent SBUF tensors ----
            # per-head augmented q/k tiles: rows 0:64 head data, row 64 holds
            # ones (kTh) / -tau (qTh) so one 65-partition matmul emits z - tau
            qTh = [sb.tile([65, S], F16, name=f"qTh{i}") for i in range(HG)]
            kTh = [sb.tile([65, S], F16, name=f"kTh{i}") for i in range(HG)]
            for i in range(HG):
                nc.vector.memset(kTh[i][64:65, :], 1.0)
            vsh = [sb.tile([P, DH], F16, name=f"vsh{i}") for i in range(NT)]
            outT = [sb.tile([P, S], F16, name=f"outT{i}") for i in range(4)]
            avg = [sb.tile([P, S], F32, name=f"avg{i}") for i in range(NT)]
            wo_sb = [sb.tile([P, S], F16, name=f"wo{i}") for i in range(4)]
            flags = sb.tile([P, HG * NT], F16)
            tau16 = sb.tile([P, HG * NT], F16)

            for i in range(4):
                # wo block for quad-rank i: int8 rows +256:320, scales +320
                wo8 = sb.tile([P, S], I8, name=f"wo8_{i}")
                nc.sync.dma_start(
                    out=wo8[:],
                    in_=wg_o[WG_ROWS * i + 256:WG_ROWS * i + 320, :].bitcast(I8))
                wsc = sb.tile([P, 1], F16, name=f"wsc_{i}")
                nc.sync.dma_start(
                    out=wsc[:], in_=wg_o[WG_ROWS * i + 320:WG_ROWS * i + 321, 0:P])
                wscf = sb.tile([P, 1], F32, name=f"wscf_{i}")
                nc.scalar.copy(out=wscf[:], in_=wsc[:])
                nc.scalar.activation(out=wo_sb[i][:], in_=wo8[:],
                                     func=ACTF.Copy, scale=wscf[:])

            # ---- phase 1: q/k/v_shared projections (scoped weights) ----
            with tc.tile_pool(name="ph1", bufs=1) as p1:
                xT_sb = [p1.tile([P, S], F16, name=f"xT{i}") for i in range(8)]
                wq_sb = [p1.tile([P, GW], F16, name=f"wq{i}") for i in range(8)]
                wk_sb = [p1.tile([P, GW], F16, name=f"wk{i}") for i in range(8)]
                wv_sb = [p1.tile([P, DH], F16, name=f"wv{i}") for i in range(8)]
                bq_sb = p1.tile([1, GW], F16)
                bk_sb = p1.tile([1, GW], F16)
                bv_sb = p1.tile([1, DH], F16)
                for i in range(8):
                    rk = WG_ROWS * (i // 2)     # quad-rank block base row
                    half = (i % 2) * 64
                    nc.sync.dma_start(out=xT_sb[i][:], in_=xg_o[i * P:(i + 1) * P, :])
                    # [64, 1024] packed rows -> [128, 512] SBUF tile (same linear order)
                    nc.sync.dma_start(out=wq_sb[i][:],
                                      in_=wg_o[rk + half:rk + half + 64, :])
                    nc.sync.dma_start(out=wk_sb[i][:],
                                      in_=wg_o[rk + 128 + half:rk + 128 + half + 64, :])
                    # [8, 1024] packed rows -> [128, 64] SBUF tile
                    h8 = (i % 2) * 8
                    nc.sync.dma_start(out=wv_sb[i][:],
                                      in_=wg_o[rk + 321 + h8:rk + 321 + h8 + 8, :])
                nc.sync.dma_start(out=bq_sb[:], in_=inb_d[RB:RB + 1, 0:GW])
                nc.sync.dma_start(out=bk_sb[:], in_=inb_d[RB:RB + 1, GW:S])
                nc.sync.dma_start(out=bv_sb[:], in_=inb_d[RBV:RBV + 1, 0:DH])

                # q/k: out[nq 128, s 512] = sum_d w[d, nq] * xT[d, s] (+ bias)
                # psum rows 0:64 -> head 2m, rows 64:128 -> head 2m+1
                for w_sb, b_sb, dst in ((wq_sb, bq_sb, qTh), (wk_sb, bk_sb, kTh)):
                    for m in range(4):          # nq tile
                        for sh in range(2):     # s half
                            ps = psB.tile([P, GW], F32, tag="psB")
                            nc.tensor.matmul(
                                ps[:], lhsT=b_sb[0:1, m * P:(m + 1) * P],
                                rhs=ones_r[0:1, :GW], start=True, stop=False)
                            for kc in range(8):
                                nc.tensor.matmul(
                                    ps[:],
                                    lhsT=w_sb[kc][:, m * P:(m + 1) * P],
                                    rhs=xT_sb[kc][:, sh * GW:(sh + 1) * GW],
                                    start=False, stop=(kc == 7))
                            nc.scalar.copy(
                                out=dst[2 * m][0:DH, sh * GW:(sh + 1) * GW],
                                in_=ps[0:DH, :])
                            nc.scalar.copy(
                                out=dst[2 * m + 1][0:DH, sh * GW:(sh + 1) * GW],
                                in_=ps[DH:P, :])

                # v_shared: out[s 128, nv 64] = sum_d xT[d, s-tile] * wv[d, nv]
                for st in range(NT):
                    ps = psO.tile([P, GW], F32, tag="psO")
                    nc.tensor.matmul(
                        ps[:, :DH], lhsT=ones_r[0:1, :P], rhs=bv_sb[0:1, :],
                        start=True, stop=False)
                    for kc in range(8):
                        nc.tensor.matmul(
                            ps[:, :DH],
                            lhsT=xT_sb[kc][:, st * P:(st + 1) * P],
                            rhs=wv_sb[kc][:], start=False, stop=(kc == 7))
                    nc.scalar.copy(out=vsh[st][:], in_=ps[:, :DH])

            zp = es.enter_context(tc.tile_pool(name="zpool", bufs=3))
            pp = es.enter_context(tc.tile_pool(name="probs", bufs=9))
            sp = es.enter_context(tc.tile_pool(name="small", bufs=6))

            # ---- phase 2: per-head attention ----
            for h in range(HG):
                # --- top-8 per query tile (j-major columns: j*8 + it) ---
                top8 = sp.tile([P, 64], F32, tag="top8")
                for it in range(NT):
                    ps = psZ.tile([P, S], F32, tag="psZ")
                    for jh in range(2):
                        nc.tensor.matmul(
                            ps[:, jh * GW:(jh + 1) * GW],
                            lhsT=qTh[h][0:DH, it * P:(it + 1) * P],
                            rhs=kTh[h][0:DH, jh * GW:(jh + 1) * GW],
                            start=True, stop=True)
                    nc.vector.max(out=top8[:, it::8], in_=ps[:])

                # --- batched tau: tj = (cumsum_j - 1)/j, tau = max_j tj ---
                cums = sp.tile([P, 64], F32, tag="cums")
                nc.vector.tensor_copy(out=cums[:, 0:8], in_=top8[:, 0:8])
                for j in range(1, 8):
                    nc.vector.tensor_tensor(
                        out=cums[:, j * 8:(j + 1) * 8],
                        in0=cums[:, (j - 1) * 8:j * 8],
                        in1=top8[:, j * 8:(j + 1) * 8], op=ALU.add)
                nc.vector.tensor_scalar_add(cums[:], cums[:], -1.0)
                nc.vector.tensor_tensor(out=cums[:], in0=cums[:], in1=recipJ[:],
                                        op=ALU.mult)
                t32 = sp.tile([P, 32], F32, tag="t32")
                for j in range(4):
                    nc.vector.tensor_tensor(
                        out=t32[:, j * 8:(j + 1) * 8],
                        in0=cums[:, 2 * j * 8:(2 * j + 1) * 8],
                        in1=cums[:, (2 * j + 1) * 8:(2 * j + 2) * 8], op=ALU.max)
                for j in range(2):
                    nc.vector.tensor_tensor(
                        out=t32[:, j * 8:(j + 1) * 8],
                        in0=t32[:, 2 * j * 8:(2 * j + 1) * 8],
                        in1=t32[:, (2 * j + 1) * 8:(2 * j + 2) * 8], op=ALU.max)
                tau8 = sp.tile([P, 8], F32, tag="tau8")
                nc.vector.tensor_tensor(out=tau8[:], in0=t32[:, 0:8],
                                        in1=t32[:, 8:16], op=ALU.max)
                nc.vector.tensor_tensor(
                    out=flags[:, h * NT:(h + 1) * NT],
                    in0=top8[:, 56:64], in1=cums[:, 56:64], op=ALU.is_gt)
                nc.scalar.copy(out=tau16[:, h * NT:(h + 1) * NT], in_=tau8[:])
                ntau8 = sp.tile([P, 8], F16, tag="ntau8")
                nc.scalar.mul(out=ntau8[:], in_=tau8[:], mul=-1.0)
                for it in range(NT):
                    nc.sync.dma_start(
                        out=qTh[h][DH:DH + 1, it * P:(it + 1) * P],
                        in_=ntau8[:, it:it + 1])

                # --- probsT = Relu(z - tau) via 65-partition matmul ---
                probs_h = []
                for jt in range(NT):
                    pr = pp.tile([P, S], F16, tag="probs")
                    probs_h.append(pr)
                    for ih in range(2):
                        ps = psB.tile([P, GW], F32, tag="psB")
                        nc.tensor.matmul(
                            ps[:],
                            lhsT=kTh[h][0:DH + 1, jt * P:(jt + 1) * P],
                            rhs=qTh[h][0:DH + 1, ih * GW:(ih + 1) * GW],
                            start=True, stop=True)
                        nc.scalar.activation(
                            out=pr[:, ih * GW:(ih + 1) * GW], in_=ps[:],
                            func=ACTF.Relu)
                    if h == 0:
                        nc.vector.tensor_copy(out=avg[jt][:], in_=probs_h[jt][:])
                    else:
                        nc.vector.tensor_tensor(
                            out=avg[jt][:], in0=avg[jt][:],
                            in1=probs_h[jt][:], op=ALU.add)

                # --- out_hT[nv, i] = sum_j vsh[j, nv] * probsT[j, i] ---
                qt, base = h // 2, (h % 2) * DH
                for ih in range(2):
                    ps = psO.tile([P, GW], F32, tag="psO")
                    for jt in range(NT):
                        nc.tensor.matmul(
                            ps[:DH, :],
                            lhsT=vsh[jt][:],
                            rhs=probs_h[jt][:, ih * GW:(ih + 1) * GW],
                            start=(jt == 0), stop=(jt == 7))
                    nc.scalar.copy(
                        out=outT[qt][base:base + DH, ih * GW:(ih + 1) * GW],
                        in_=ps[:DH, :])

            # ---- output pair-reduce staging: rows 0:1024 x_outT, 1024:2048 avg/H
            rs_i = dram.tile([2 * S, S], F16)
            rs_o = dram.tile([S, S], F16)

            for jt in range(NT):
                a16 = zp.tile([P, S], F16, tag="a16")
                nc.scalar.mul(out=a16[:], in_=avg[jt][:], mul=1.0 / H)
                nc.sync.dma_start(out=rs_i[S + jt * P:S + (jt + 1) * P, :], in_=a16[:])

            # ---- phase 3: x_outT[dcol, i] = sum_nc wo[nc, dcol] outT[nc, i] ----
            for m in range(8):
                xo = zp.tile([P, S], F16, tag="xo")
                for ih in range(2):
                    ps = psB.tile([P, GW], F32, tag="psB")
                    for kc in range(4):
                        nc.tensor.matmul(
                            ps[:],
                            lhsT=wo_sb[kc][:, m * P:(m + 1) * P],
                            rhs=outT[kc][:, ih * GW:(ih + 1) * GW],
                            start=(kc == 0), stop=(kc == 3))
                    nc.scalar.copy(out=xo[:, ih * GW:(ih + 1) * GW], in_=ps[:])
                nc.sync.dma_start(out=rs_i[m * P:(m + 1) * P, :], in_=xo[:])

            # rank g=0 receives sum(x_outT), rank g=1 receives sum(avgT)/H
            nc.gpsimd.collective_compute(
                "ReduceScatter", ALU.add, replica_groups=PAIRS,
                ins=[rs_i.opt()], outs=[rs_o.opt()])

            # row-quantize the fp16 payload to int8: q = round(v * QMAX/rowmax)
            scales = sb.tile([P, NT], F16, name="scales")
            for t in range(NT):
                qin = zp.tile([P, S], F16, tag="qin")
                nc.sync.dma_start(out=qin[:], in_=rs_o[t * P:(t + 1) * P, :])
                rmax = sp.tile([P, 1], F32, tag="rmax")
                rmin = sp.tile([P, 1], F32, tag="rmin")
                nc.vector.tensor_reduce(out=rmax[:], in_=qin[:], axis=AX.X,
                                        op=ALU.max)
                nc.vector.tensor_reduce(out=rmin[:], in_=qin[:], axis=AX.X,
                                        op=ALU.min)
                nc.scalar.mul(out=rmin[:], in_=rmin[:], mul=-1.0)
                nc.vector.tensor_tensor(out=rmax[:], in0=rmax[:], in1=rmin[:],
                                        op=ALU.max)
                nc.vector.tensor_scalar(out=rmax[:], in0=rmax[:], scalar1=1e-6,
                                        scalar2=None, op0=ALU.max)
                nc.scalar.copy(out=scales[:, t:t + 1], in_=rmax[:])
                rsc = sp.tile([P, 1], F32, tag="rsc")
                nc.vector.reciprocal(out=rsc[:], in_=rmax[:])
                nc.vector.tensor_scalar(out=rsc[:], in0=rsc[:], scalar1=QMAX,
                                        scalar2=None, op0=ALU.mult)
                q8 = zp.tile([P, S], I8, tag="q8")
                nc.scalar.activation(out=q8[:], in_=qin[:], func=ACTF.Copy,
                                     scale=rsc[:])
                nc.sync.dma_start(out=outb_d[t * P:(t + 1) * P, :], in_=q8[:])

            nc.sync.dma_start(out=outb_d[OSCALE:OSCALE + 2, :],
                              in_=scales[:].bitcast(I8))
            nc.sync.dma_start(out=outb_d[OTAU:OTAU + 16, :],
                              in_=tau16[:].bitcast(I8))
            nc.sync.dma_start(out=outb_d[OFLAG:OFLAG + 16, :],
                              in_=flags[:].bitcast(I8))

    nc.compile()
    return nc


def _sparsemax_row(z):
    zs = -np.sort(-z)
    cs = np.cumsum(zs)
    k = np.arange(1, z.shape[0] + 1)
    supp = (1.0 + k * zs) > cs
    ksz = int(supp.sum())
    tau = (cs[ksz - 1] - 1.0) / ksz
    return np.maximum(z - tau, 0.0)


def _make_in_maps(x, Wq, bq, Wk, bk, Wv, bv, Wo, bo):
    wv_sh = Wv.reshape(D, H, DH).mean(axis=1)          # (D, 64)
    bv_sh = bv.reshape(H, DH).mean(axis=0)             # (64,)
    in_maps = []
    for c in range(N_CORES):
        b_idx, g = c // 2, c % 2
        cols = slice(g * GW, (g + 1) * GW)
        q4 = slice(b_idx * (D // 4), (b_idx + 1) * (D // 4))
        blob = np.zeros((IN_ROWS, S), np.float16)
        blob[RX:RX + GW] = x[b_idx][:, g * GW:(g + 1) * GW].T
        blob[RWQ:RWQ + 128] = (Wq[q4, cols] * 0.125).astype(np.float16).reshape(128, S)
        blob[RWK:RWK + 128] = Wk[q4, cols].astype(np.float16).reshape(128, S)
        wo_blk = Wo[g * GW + b_idx * P:g * GW + (b_idx + 1) * P, :]
        wmax = np.maximum(np.abs(wo_blk).max(axis=1), 1e-12)
        wo8 = np.clip(np.round(wo_blk * (QMAX / wmax[:, None])),
                      -127, 127).astype(np.int8)
        blob[RWO:RWO + 64] = wo8.reshape(64, 2 * S).view(np.float16)
        blob[RWOS, 0:P] = (wmax / QMAX).astype(np.float16)
        blob[RWV:RWV + 16] = wv_sh[q4, :].astype(np.float16).reshape(16, S)
        blob[RB, 0:GW] = bq[cols] * 0.125
        blob[RB, GW:S] = bk[cols]
        blob[RBV, 0:DH] = bv_sh
        in_maps.append({"inb": blob})
    return in_maps, wv_sh, bv_sh


def kernel(x, Wq, bq, Wk, bk, Wv, bv, Wo, bo):
    x = np.asarray(x, dtype=np.float32)
    Wq = np.asarray(Wq, dtype=np.float32); bq = np.asarray(bq, dtype=np.float32)
    Wk = np.asarray(Wk, dtype=np.float32); bk = np.asarray(bk, dtype=np.float32)
    Wv = np.asarray(Wv, dtype=np.float32); bv = np.asarray(bv, dtype=np.float32)
    Wo = np.asarray(Wo, dtype=np.float32); bo = np.asarray(bo, dtype=np.float32)

    if "nc" not in _cached:
        _cached["nc"] = _build()
    nc = _cached["nc"]

    in_maps, wv_sh, bv_sh = _make_in_maps(x, Wq, bq, Wk, bk, Wv, bv, Wo, bo)
    # The axon tunnel occasionally drops mid-execute ("worker hung up").
    # Reset the PJRT backend, disable the compile cache (in case a cached
    # executable is implicated), and retry -- the NEFF compile is file-cached,
    # so a retry only repays the execute cost.
    res = None
    for attempt in range(5):
        try:
            res = run_bass_kernel_spmd(nc, in_maps, list(range(N_CORES)))
            break
        except Exception:
            if attempt == 4:
                raise
            _set_compile_cache(False)
            import os as _os
            _os.environ["NEURON_RT_RESET_CORES"] = "1"
            try:
                import jax.extend as _jex
                _jex.backend.clear_backends()
            except Exception:
                pass
            import time as _time
            _time.sleep(5.0 + 15.0 * attempt)
    r = res.results

    def _dequant(blob):
        scal = blob[OSCALE:OSCALE + 2].reshape(-1).view(np.float16)
        rowscale = (scal.reshape(P, NT).T.reshape(-1).astype(np.float32) / QMAX)
        return blob[0:S, :].astype(np.float32) * rowscale[:, None]

    x_out = np.empty((B, S, D), dtype=np.float32)
    avg = np.empty((B, S, S), dtype=np.float32)
    for b_idx in range(B):
        x_out[b_idx] = _dequant(r[2 * b_idx]["outb"]).T + bo
        avg[b_idx] = _dequant(r[2 * b_idx + 1]["outb"]).T

    # ---- host fixup of rows with sparsemax support >= 8 ----
    flagged = []   # (b, head, i, tau_dev)
    for c in range(N_CORES):
        fl = r[c]["outb"][OFLAG:OFLAG + 16, :].reshape(-1).view(
            np.float16).reshape(P, HG * NT)
        taus = r[c]["outb"][OTAU:OTAU + 16, :].reshape(-1).view(
            np.float16).reshape(P, HG * NT)
        ps, gs = np.nonzero(fl > 0.5)
        for p, g64 in zip(ps, gs):
            head = (c % 2) * HG + g64 // NT
            i = (g64 % NT) * P + int(p)
            flagged.append((c // 2, head, i, float(taus[p, g64])))

    if flagged:
        bs_needed = sorted({f[0] for f in flagged})
        qkv_cache = {}
        for b_idx in bs_needed:
            qkv_cache[b_idx] = (
                x[b_idx] @ Wq + bq,
                x[b_idx] @ Wk + bk,
                x[b_idx] @ wv_sh + bv_sh,
            )
        scale = 1.0 / np.sqrt(DH)
        for b_idx, head, i, tau_dev in flagged:
            qb, kb, vb = qkv_cache[b_idx]
            hc = slice(head * DH, (head + 1) * DH)
            z = (qb[i, hc] @ kb[:, hc].T) * scale          # (S,)
            probs_new = _sparsemax_row(z)
            probs_old = np.maximum(z - tau_dev, 0.0)
            delta = probs_new - probs_old
            avg[b_idx, i, :] += delta / H
            x_out[b_idx, i, :] += (delta @ vb) @ Wo[hc, :]

    return x_out, avg


# revision 17
# speedup vs baseline: 1.0222x; 1.0222x over previous
"""InterpretableMultiHeadAttention kernel for 8 Trainium2 NeuronCores.

Math (per batch b): q/k = x@Wq/k + b; per-head logits = q_h k_h^T/sqrt(dh);
probs = sparsemax(logits); shared V = head-mean of v (linear -> fold into a
(D, dh) weight); out = concat_h(probs_h @ v_shared) @ Wo + bo;
avg_attention = mean_h probs.

Sharding: core c handles batch b=c//2, head-group g=c%2 (8 of 16 heads).

Wire-format optimization (the metric is dominated by host<->device transfer
over the axon tunnel, ~19 ms/MB each way plus ~10 ms per array):
  * everything on the wire is fp16 (intermediate math stays f32 in PSUM);
  * each core uploads ONE packed blob holding only a disjoint shard of
    x^T / Wq / Wk / Wo / v_shared; full tensors are reassembled on-device
    with two AllGather collectives (pair groups for x, quad groups for the
    weights);
  * the two big outputs are combined by a single pair-ReduceScatter whose
    rank split hands core (b,0) the summed x_out^T[b] and core (b,1) the
    summed avg^T[b], so each core downloads ONE packed blob and the
    head-group partial sums never cross the tunnel;
  * the downloaded payload is row-quantized to int8 on device (scale =
    126/rowmax per row, fp16 row-maxes shipped alongside) -- halves both the
    download and the donated-zero upload for ~0.3% added output error.

Everything on-device is computed transposed (queries on the free axis) so
every matmul consumes natural layouts; the host transposes the two big
outputs at the end.

Sparsemax per query row: top-8 extraction (nc.vector.max reading the logits
PSUM directly, sorted desc), closed-form tau* = max_j (cumsum_j - 1)/j over
the sorted prefix, with the cumsum/argmax math batched across all 8 query
tiles of a head (j-major [128, 64] layout).  Rows whose support size is >= 8
are flagged (z_8 > t_8) and corrected exactly on the host (~1% of rows for
this distribution).  -tau is scattered into row 64 of the per-head augmented
q tile (k tile row 64 holds ones), so a single 65-partition matmul emits
z - tau and the final probs come out of one fused Relu(PSUM) pass.

The runtime charges ~44 us of dispatch overhead per instruction, so the
device program batches scalar work aggressively (~1300 instructions total);
engine time itself is ~0.5 ms (TimelineSim).
"""

import sys

sys.path.insert(0, "/opt/trn_rl_repo")

import jax


def _set_compile_cache(enabled):
    # run_bass_via_pjrt builds a fresh jit(shard_map(...)) per call; the
    # persistent cache lets warm calls load the XLA executable from disk
    # (~150 ms/call here).  Disabled on retry after a tunnel failure so the
    # fallback path recompiles from scratch.
    try:
        jax.config.update("jax_compilation_cache_dir",
                          "/tmp/jax_cc_cache" if enabled else None)
        jax.config.update("jax_persistent_cache_min_compile_time_secs", 0.0)
        jax.config.update("jax_persistent_cache_min_entry_size_bytes", -1)
    except Exception:
        pass


_set_compile_cache(True)

import numpy as np
from contextlib import ExitStack

import concourse.bacc as bacc
import concourse.mybir as mybir
import concourse.tile as tile
from concourse.bass_utils import run_bass_kernel_spmd

F32 = mybir.dt.float32
F16 = mybir.dt.float16
I8 = mybir.dt.int8
AX = mybir.AxisListType
ALU = mybir.AluOpType
ACTF = mybir.ActivationFunctionType

N_CORES = 8
P = 128
B, S, D = 4, 1024, 1024
H = 16                      # total heads
HG = 8                      # heads per core (head-group)
DH = D // H                 # 64
GW = D // 2                 # 512 = per-group projection width
NT = S // P                 # 8 i/j tiles

# input blob row offsets (width 1024, fp16)
RX = 0          # xTh               [512, 1024]
RWQ = 512       # wq   [256,512] -> [128, 1024]
RWK = 640       # wk   [256,512] -> [128, 1024]
RWO = 768       # wo int8 [128,1024] -> [64, 1024] f16-packed
RWOS = 832      # wo row-maxes/126 f16: [1, 1024] (cols 0:128 used)
RWV = 833       # wv   [256, 64] -> [ 16, 1024]
RB = 849        # row: [bq*0.125 | bk]
RBV = 850       # row: [bv_sh pad]
IN_ROWS = 851
WG_ROWS = RB - RWQ          # 337 rows gathered per quad rank

# output blob row offsets (width 1024, int8): rows 0:1024 row-quantized
# payload, 2 rows fp16 row-maxes (bitcast), then tau/flag fp16 (bitcast)
OSCALE = 1024   # rowmax [128, 8] f16 -> [2, 1024] i8
OTAU = 1026     # tau  [128, 64] f16 -> [16, 1024] i8
OFLAG = 1042    # flag [128, 64] f16 -> [16, 1024] i8
OUT_ROWS = 1058
QMAX = 126.0

PAIRS = [[0, 1], [2, 3], [4, 5], [6, 7]]     # {batch} x {head-group g}
QUADS = [[0, 2, 4, 6], [1, 3, 5, 7]]         # same g across batches
_cached = {}


def _build():
    nc = bacc.Bacc("TRN2", target_bir_lowering=False, debug=False,
                   num_devices=N_CORES)

    inb_d = nc.dram_tensor("inb", [IN_ROWS, S], F16, kind="ExternalInput").ap()
    outb_d = nc.dram_tensor("outb", [OUT_ROWS, S], I8, kind="ExternalOutput").ap()

    with tile.TileContext(nc) as tc, ExitStack() as es:
        dram = es.enter_context(tc.tile_pool(name="dram", bufs=1, space="DRAM"))
        sb = es.enter_context(tc.tile_pool(name="persist", bufs=1))
        psZ = es.enter_context(tc.tile_pool(name="psZ", bufs=2, space="PSUM"))
        psB = es.enter_context(tc.tile_pool(name="psB", bufs=2, space="PSUM"))
        psO = es.enter_context(tc.tile_pool(name="psO", bufs=2, space="PSUM"))
        if True:
            # ---- input gather collectives (DRAM bounce buffers) ----
            xg_i = dram.tile([GW, S], F16)
            xg_o = dram.tile([D, S], F16)          # full xT
            wg_i = dram.tile([WG_ROWS, S], F16)
            wg_o = dram.tile([4 * WG_ROWS, S], F16)  # 4 rank blocks of weights

            nc.gpsimd.dma_start(xg_i[:], inb_d[RX:RX + GW, :])
            nc.gpsimd.dma_start(wg_i[:], inb_d[RWQ:RB, :])
            nc.gpsimd.collective_compute(
                "AllGather", ALU.bypass, replica_groups=PAIRS,
                ins=[xg_i.opt()], outs=[xg_o.opt()])
            nc.gpsimd.collective_compute(
                "AllGather", ALU.bypass, replica_groups=QUADS,
                ins=[wg_i.opt()], outs=[wg_o.opt()])

            # ---- constants ----
            ones_r = sb.tile([1, S], F16)
            nc.vector.memset(ones_r[:], 1.0)
            recipJ = sb.tile([P, 64], F32)       # j-major: block j = 1/(j+1)
            for j in range(8):
                nc.vector.memset(recipJ[:, j * 8:(j + 1) * 8], 1.0 / (j + 1))

            # ---- persistent SBUF tensors ----
            # per-head augmented q/k tiles: rows 0:64 head data, row 64 holds
            # ones (kTh) / -tau (qTh) so one 65-partition matmul emits z - tau
            qTh = [sb.tile([65, S], F16, name=f"qTh{i}") for i in range(HG)]
            kTh = [sb.tile([65, S], F16, name=f"kTh{i}") for i in range(HG)]
            for i in range(HG):
                nc.vector.memset(kTh[i][64:65, :], 1.0)
            vsh = [sb.tile([P, DH], F16, name=f"vsh{i}") for i in range(NT)]
            outT = [sb.tile([P, S], F16, name=f"outT{i}") for i in range(4)]
            avg = [sb.tile([P, S], F32, name=f"avg{i}") for i in range(NT)]
            wo_sb = [sb.tile([P, S], F16, name=f"wo{i}") for i in range(4)]
            flags = sb.tile([P, HG * NT], F16)
            tau16 = sb.tile([P, HG * NT], F16)

            for i in range(4):
                # wo block for quad-rank i: int8 rows +256:320, scales +320
                wo8 = sb.tile([P, S], I8, name=f"wo8_{i}")
                nc.sync.dma_start(
                    out=wo8[:],
                    in_=wg_o[WG_ROWS * i + 256:WG_ROWS * i + 320, :].bitcast(I8))
                wsc = sb.tile([P, 1], F16, name=f"wsc_{i}")
                nc.sync.dma_start(
                    out=wsc[:], in_=wg_o[WG_ROWS * i + 320:WG_ROWS * i + 321, 0:P])
                wscf = sb.tile([P, 1], F32, name=f"wscf_{i}")
                nc.scalar.copy(out=wscf[:], in_=wsc[:])
                nc.scalar.activation(out=wo_sb[i][:], in_=wo8[:],
                                     func=ACTF.Copy, scale=wscf[:])

            # ---- phase 1: q/k/v_shared projections (scoped weights) ----
            with tc.tile_pool(name="ph1", bufs=1) as p1:
                xT_sb = [p1.tile([P, S], F16, name=f"xT{i}") for i in range(8)]
                wq_sb = [p1.tile([P, GW], F16, name=f"wq{i}") for i in range(8)]
                wk_sb = [p1.tile([P, GW], F16, name=f"wk{i}") for i in range(8)]
                wv_sb = [p1.tile([P, DH], F16, name=f"wv{i}") for i in range(8)]
                bq_sb = p1.tile([1, GW], F16)
                bk_sb = p1.tile([1, GW], F16)
                bv_sb = p1.tile([1, DH], F16)
                for i in range(8):
                    rk = WG_ROWS * (i // 2)     # quad-rank block base row
                    half = (i % 2) * 64
                    nc.sync.dma_start(out=xT_sb[i][:], in_=xg_o[i * P:(i + 1) * P, :])
                    # [64, 1024] packed rows -> [128, 512] SBUF tile (same linear order)
                    nc.sync.dma_start(out=wq_sb[i][:],
                                      in_=wg_o[rk + half:rk + half + 64, :])
                    nc.sync.dma_start(out=wk_sb[i][:],
                                      in_=wg_o[rk + 128 + half:rk + 128 + half + 64, :])
                    # [8, 1024] packed rows -> [128, 64] SBUF tile
                    h8 = (i % 2) * 8
                    nc.sync.dma_start(out=wv_sb[i][:],
                                      in_=wg_o[rk + 321 + h8:rk + 321 + h8 + 8, :])
                nc.sync.dma_start(out=bq_sb[:], in_=inb_d[RB:RB + 1, 0:GW])
                nc.sync.dma_start(out=bk_sb[:], in_=inb_d[RB:RB + 1, GW:S])
                nc.sync.dma_start(out=bv_sb[:], in_=inb_d[RBV:RBV + 1, 0:DH])

                # q/k: out[nq 128, s 512] = sum_d w[d, nq] * xT[d, s] (+ bias)
                # psum rows 0:64 -> head 2m, rows 64:128 -> head 2m+1
                for w_sb, b_sb, dst in ((wq_sb, bq_sb, qTh), (wk_sb, bk_sb, kTh)):
                    for m in range(4):          # nq tile
                        for sh in range(2):     # s half
                            ps = psB.tile([P, GW], F32, tag="psB")
                            nc.tensor.matmul(
                                ps[:], lhsT=b_sb[0:1, m * P:(m + 1) * P],
                                rhs=ones_r[0:1, :GW], start=True, stop=False)
                            for kc in range(8):
                                nc.tensor.matmul(
                                    ps[:],
                                    lhsT=w_sb[kc][:, m * P:(m + 1) * P],
                                    rhs=xT_sb[kc][:, sh * GW:(sh + 1) * GW],
                                    start=False, stop=(kc == 7))
                            nc.scalar.copy(
                                out=dst[2 * m][0:DH, sh * GW:(sh + 1) * GW],
                                in_=ps[0:DH, :])
                            nc.scalar.copy(
                                out=dst[2 * m + 1][0:DH, sh * GW:(sh + 1) * GW],
                                in_=ps[DH:P, :])

                # v_shared: out[s 128, nv 64] = sum_d xT[d, s-tile] * wv[d, nv]
                for st in range(NT):
                    ps = psO.tile([P, GW], F32, tag="psO")
                    nc.tensor.matmul(
                        ps[:, :DH], lhsT=ones_r[0:1, :P], rhs=bv_sb[0:1, :],
                        start=True, stop=False)
                    for kc in range(8):
                        nc.tensor.matmul(
                            ps[:, :DH],
                            lhsT=xT_sb[kc][:, st * P:(st + 1) * P],
                            rhs=wv_sb[kc][:], start=False, stop=(kc == 7))
                    nc.scalar.copy(out=vsh[st][:], in_=ps[:, :DH])

            zp = es.enter_context(tc.tile_pool(name="zpool", bufs=3))
            pp = es.enter_context(tc.tile_pool(name="probs", bufs=9))
            sp = es.enter_context(tc.tile_pool(name="small", bufs=6))

            # ---- phase 2: per-head attention ----
            for h in range(HG):
                # --- top-8 per query tile (j-major columns: j*8 + it) ---
                top8 = sp.tile([P, 64], F32, tag="top8")
                for it in range(NT):
                    ps = psZ.tile([P, S], F32, tag="psZ")
                    for jh in range(2):
                        nc.tensor.matmul(
                            ps[:, jh * GW:(jh + 1) * GW],
                            lhsT=qTh[h][0:DH, it * P:(it + 1) * P],
                            rhs=kTh[h][0:DH, jh * GW:(jh + 1) * GW],
                            start=True, stop=True)
                    nc.vector.max(out=top8[:, it::8], in_=ps[:])

                # --- batched tau: tj = (cumsum_j - 1)/j, tau = max_j tj ---
                cums = sp.tile([P, 64], F32, tag="cums")
                nc.vector.tensor_copy(out=cums[:, 0:8], in_=top8[:, 0:8])
                for j in range(1, 8):
                    nc.vector.tensor_tensor(
                        out=cums[:, j * 8:(j + 1) * 8],
                        in0=cums[:, (j - 1) * 8:j * 8],
                        in1=top8[:, j * 8:(j + 1) * 8], op=ALU.add)
                nc.vector.tensor_scalar_add(cums[:], cums[:], -1.0)
                nc.vector.tensor_tensor(out=cums[:], in0=cums[:], in1=recipJ[:],
                                        op=ALU.mult)
                t32 = sp.tile([P, 32], F32, tag="t32")
                for j in range(4):
                    nc.vector.tensor_tensor(
                        out=t32[:, j * 8:(j + 1) * 8],
                        in0=cums[:, 2 * j * 8:(2 * j + 1) * 8],
                        in1=cums[:, (2 * j + 1) * 8:(2 * j + 2) * 8], op=ALU.max)
                for j in range(2):
                    nc.vector.tensor_tensor(
                        out=t32[:, j * 8:(j + 1) * 8],
                        in0=t32[:, 2 * j * 8:(2 * j + 1) * 8],
                        in1=t32[:, (2 * j + 1) * 8:(2 * j + 2) * 8], op=ALU.max)
                tau8 = sp.tile([P, 8], F32, tag="tau8")
                nc.vector.tensor_tensor(out=tau8[:], in0=t32[:, 0:8],
                                        in1=t32[:, 8:16], op=ALU.max)
                nc.vector.tensor_tensor(
                    out=flags[:, h * NT:(h + 1) * NT],
                    in0=top8[:, 56:64], in1=cums[:, 56:64], op=ALU.is_gt)
                nc.scalar.copy(out=tau16[:, h * NT:(h + 1) * NT], in_=tau8[:])
                ntau8 = sp.tile([P, 8], F16, tag="ntau8")
                nc.scalar.mul(out=ntau8[:], in_=tau8[:], mul=-1.0)
                for it in range(NT):
                    nc.sync.dma_start(
                        out=qTh[h][DH:DH + 1, it * P:(it + 1) * P],
                        in_=ntau8[:, it:it + 1])

                # --- probsT = Relu(z - tau) via 65-partition matmul ---
                probs_h = []
                for jt in range(NT):
                    pr = pp.tile([P, S], F16, tag="probs")
                    probs_h.append(pr)
                    for ih in range(2):
                        ps = psB.tile([P, GW], F32, tag="psB")
                        nc.tensor.matmul(
                            ps[:],
                            lhsT=kTh[h][0:DH + 1, jt * P:(jt + 1) * P],
                            rhs=qTh[h][0:DH + 1, ih * GW:(ih + 1) * GW],
                            start=True, stop=True)
                        nc.scalar.activation(
                            out=pr[:, ih * GW:(ih + 1) * GW], in_=ps[:],
                            func=ACTF.Relu)
                    if h == 0:
                        nc.vector.tensor_copy(out=avg[jt][:], in_=probs_h[jt][:])
                    else:
                        nc.vector.tensor_tensor(
                            out=avg[jt][:], in0=avg[jt][:],
                            in1=probs_h[jt][:], op=ALU.add)

                # --- out_hT[nv, i] = sum_j vsh[j, nv] * probsT[j, i] ---
                qt, base = h // 2, (h % 2) * DH
                for ih in range(2):
                    ps = psO.tile([P, GW], F32, tag="psO")
                    for jt in range(NT):
                        nc.tensor.matmul(
                            ps[:DH, :],
                            lhsT=vsh[jt][:],
                            rhs=probs_h[jt][:, ih * GW:(ih + 1) * GW],
                            start=(jt == 0), stop=(jt == 7))
                    nc.scalar.copy(
                        out=outT[qt][base:base + DH, ih * GW:(ih + 1) * GW],
                        in_=ps[:DH, :])

            # ---- output pair-reduce staging: rows 0:1024 x_outT, 1024:2048 avg/H
            rs_i = dram.tile([2 * S, S], F16)
            rs_o = dram.tile([S, S], F16)

            for jt in range(NT):
                a16 = zp.tile([P, S], F16, tag="a16")
                nc.scalar.mul(out=a16[:], in_=avg[jt][:], mul=1.0 / H)
                nc.sync.dma_start(out=rs_i[S + jt * P:S + (jt + 1) * P, :], in_=a16[:])

            # ---- phase 3: x_outT[dcol, i] = sum_nc wo[nc, dcol] outT[nc, i] ----
            for m in range(8):
                xo = zp.tile([P, S], F16, tag="xo")
                for ih in range(2):
                    ps = psB.tile([P, GW], F32, tag="psB")
                    for kc in range(4):
                        nc.tensor.matmul(
                            ps[:],
                            lhsT=wo_sb[kc][:, m * P:(m + 1) * P],
                            rhs=outT[kc][:, ih * GW:(ih + 1) * GW],
                            start=(kc == 0), stop=(kc == 3))
                    nc.scalar.copy(out=xo[:, ih * GW:(ih + 1) * GW], in_=ps[:])
                nc.sync.dma_start(out=rs_i[m * P:(m + 1) * P, :], in_=xo[:])

            # rank g=0 receives sum(x_outT), rank g=1 receives sum(avgT)/H
            nc.gpsimd.collective_compute(
                "ReduceScatter", ALU.add, replica_groups=PAIRS,
                ins=[rs_i.opt()], outs=[rs_o.opt()])

            # row-quantize the fp16 payload to int8: q = round(v * QMAX/rowmax)
            scales = sb.tile([P, NT], F16, name="scales")
            for t in range(NT):
                qin = zp.tile([P, S], F16, tag="qin")
                nc.sync.dma_start(out=qin[:], in_=rs_o[t * P:(t + 1) * P, :])
                rmax = sp.tile([P, 1], F32, tag="rmax")
                rmin = sp.tile([P, 1], F32, tag="rmin")
                nc.vector.tensor_reduce(out=rmax[:], in_=qin[:], axis=AX.X,
                                        op=ALU.max)
                nc.vector.tensor_reduce(out=rmin[:], in_=qin[:], axis=AX.X,
                                        op=ALU.min)
                nc.scalar.mul(out=rmin[:], in_=rmin[:], mul=-1.0)
                nc.vector.tensor_tensor(out=rmax[:], in0=rmax[:], in1=rmin[:],
                                        op=ALU.max)
                nc.vector.tensor_scalar(out=rmax[:], in0=rmax[:], scalar1=1e-6,
                                        scalar2=None, op0=ALU.max)
                nc.scalar.copy(out=scales[:, t:t + 1], in_=rmax[:])
                rsc = sp.tile([P, 1], F32, tag="rsc")
                nc.vector.reciprocal(out=rsc[:], in_=rmax[:])
                nc.vector.tensor_scalar(out=rsc[:], in0=rsc[:], scalar1=QMAX,
                                        scalar2=None, op0=ALU.mult)
                q8 = zp.tile([P, S], I8, tag="q8")
                nc.scalar.activation(out=q8[:], in_=qin[:], func=ACTF.Copy,
                                     scale=rsc[:])
                nc.sync.dma_start(out=outb_d[t * P:(t + 1) * P, :], in_=q8[:])

            nc.sync.dma_start(out=outb_d[OSCALE:OSCALE + 2, :],
                              in_=scales[:].bitcast(I8))
            nc.sync.dma_start(out=outb_d[OTAU:OTAU + 16, :],
                              in_=tau16[:].bitcast(I8))
            nc.sync.dma_start(out=outb_d[OFLAG:OFLAG + 16, :],
                              in_=flags[:].bitcast(I8))

    nc.compile()
    return nc


def _sparsemax_row(z):
    zs = -np.sort(-z)
    cs = np.cumsum(zs)
    k = np.arange(1, z.shape[0] + 1)
    supp = (1.0 + k * zs) > cs
    ksz = int(supp.sum())
    tau = (cs[ksz - 1] - 1.0) / ksz
    return np.maximum(z - tau, 0.0)


def _make_in_maps(x, Wq, bq, Wk, bk, Wv, bv, Wo, bo):
    wv_sh = Wv.reshape(D, H, DH).mean(axis=1)          # (D, 64)
    bv_sh = bv.reshape(H, DH).mean(axis=0)             # (64,)
    in_maps = []
    for c in range(N_CORES):
        b_idx, g = c // 2, c % 2
        cols = slice(g * GW, (g + 1) * GW)
        q4 = slice(b_idx * (D // 4), (b_idx + 1) * (D // 4))
        blob = np.zeros((IN_ROWS, S), np.float16)
        blob[RX:RX + GW] = x[b_idx][:, g * GW:(g + 1) * GW].T
        blob[RWQ:RWQ + 128] = (Wq[q4, cols] * 0.125).astype(np.float16).reshape(128, S)
        blob[RWK:RWK + 128] = Wk[q4, cols].astype(np.float16).reshape(128, S)
        wo_blk = Wo[g * GW + b_idx * P:g * GW + (b_idx + 1) * P, :]
        wmax = np.maximum(np.abs(wo_blk).max(axis=1), 1e-12)
        wo8 = np.clip(np.round(wo_blk * (QMAX / wmax[:, None])),
                      -127, 127).astype(np.int8)
        blob[RWO:RWO + 64] = wo8.reshape(64, 2 * S).view(np.float16)
        blob[RWOS, 0:P] = (wmax / QMAX).astype(np.float16)
        blob[RWV:RWV + 16] = wv_sh[q4, :].astype(np.float16).reshape(16, S)
        blob[RB, 0:GW] = bq[cols] * 0.125
        blob[RB, GW:S] = bk[cols]
        blob[RBV, 0:DH] = bv_sh
        in_maps.append({"inb": blob})
    return in_maps, wv_sh, bv_sh


def kernel(x, Wq, bq, Wk, bk, Wv, bv, Wo, bo):
    x = np.asarray(x, dtype=np.float32)
    Wq = np.asarray(Wq, dtype=np.float32); bq = np.asarray(bq, dtype=np.float32)
    Wk = np.asarray(Wk, dtype=np.float32); bk = np.asarray(bk, dtype=np.float32)
    Wv = np.asarray(Wv, dtype=np.float32); bv = np.asarray(bv, dtype=np.float32)
    Wo = np.asarray(Wo, dtype=np.float32); bo = np.asarray(bo, dtype=np.float32)

    if "nc" not in _cached:
        _cached["nc"] = _build()
    nc = _cached["nc"]

    in_maps, wv_sh, bv_sh = _make_in_maps(x, Wq, bq, Wk, bk, Wv, bv, Wo, bo)
    # The axon tunnel occasionally drops mid-execute ("worker hung up").
    # Reset the PJRT backend, disable the compile cache (in case a cached
    # executable is implicated), and retry -- the NEFF compile is file-cached,
    # so a retry only repays the execute cost.
    res = None
    for attempt in range(6):
        try:
            res = run_bass_kernel_spmd(nc, in_maps, list(range(N_CORES)))
            break
        except Exception:
            if attempt == 5:
                raise
            _set_compile_cache(False)
            import os as _os
            _os.environ["NEURON_RT_RESET_CORES"] = "1"
            try:
                import jax.extend as _jex
                _jex.backend.clear_backends()
            except Exception:
                pass
            import time as _time
            _time.sleep(5.0 + 20.0 * attempt)
    r = res.results

    def _dequant(blob):
        scal = blob[OSCALE:OSCALE + 2].reshape(-1).view(np.float16)
        rowscale = (scal.reshape(P, NT).T.reshape(-1).astype(np.float32) / QMAX)
        return blob[0:S, :].astype(np.float32) * rowscale[:, None]

    x_out = np.empty((B, S, D), dtype=np.float32)
    avg = np.empty((B, S, S), dtype=np.float32)
    for b_idx in range(B):
        x_out[b_idx] = _dequant(r[2 * b_idx]["outb"]).T + bo
        avg[b_idx] = _dequant(r[2 * b_idx + 1]["outb"]).T

    # ---- host fixup of rows with sparsemax support >= 8 ----
    flagged = []   # (b, head, i, tau_dev)
    for c in range(N_CORES):
        fl = r[c]["outb"][OFLAG:OFLAG + 16, :].reshape(-1).view(
            np.float16).reshape(P, HG * NT)
        taus = r[c]["outb"][OTAU:OTAU + 16, :].reshape(-1).view(
            np.float16).reshape(P, HG * NT)
        ps, gs = np.nonzero(fl > 0.5)
        for p, g64 in zip(ps, gs):
            head = (c % 2) * HG + g64 // NT
            i = (g64 % NT) * P + int(p)
            flagged.append((c // 2, head, i, float(taus[p, g64])))

    if flagged:
        bs_needed = sorted({f[0] for f in flagged})
        qkv_cache = {}
        for b_idx in bs_needed:
            qkv_cache[b_idx] = (
                x[b_idx] @ Wq + bq,
                x[b_idx] @ Wk + bk,
                x[b_idx] @ wv_sh + bv_sh,
            )
        scale = 1.0 / np.sqrt(DH)
        for b_idx, head, i, tau_dev in flagged:
            qb, kb, vb = qkv_cache[b_idx]
            hc = slice(head * DH, (head + 1) * DH)
            z = (qb[i, hc] @ kb[:, hc].T) * scale          # (S,)
            probs_new = _sparsemax_row(z)
            probs_old = np.maximum(z - tau_dev, 0.0)
            delta = probs_new - probs_old
            avg[b_idx, i, :] += delta / H
            x_out[b_idx, i, :] += (delta @ vb) @ Wo[hc, :]

    return x_out, avg
